# revision 3
# baseline (speedup 1.0000x reference)
"""Trainium2 Bass kernel for nn_Attention_18433999635238.

Reference computation (b=4, n=4096, dx=256, de=128, d=384):
    xe = concat([x, e], -1)                  # [b, n, d]
    q = xe @ Wq.T ; k = xe @ Wk.T            # [b, n, d]
    attn = tanh(q @ k.T) * d**-0.5           # [b, n, n]
    out = attn @ x + x                       # [b, n, dx]
    cur = out[:, :, -1:]
    for _ in range(pred_step - 1): cur = attn @ cur + cur ; append
    return concat([out, *extras], -1), attn

Sharding: 8 cores = 4 batches x 2 row-halves. Each core computes a
[2048, 4096] row-block of attn and the matching [2048, 256] rows of the
AV+residual output. The tiny autoregressive tail (3 matvec columns,
0.4% of FLOPs) is finished on host from the returned attn.

Device pipeline per core (all matmuls in float32r: full-rate fp32 with
11-bit mantissa; inputs pre-rounded on host):
    kT = WkT.T @ xeT   (d x 4096)        qT = WqT.T @ xeTl  (d x 2048)
    per (row-tile j of 128, m-block i of 1024):
      psum_L = qT_j.T @ kT_i             # natural attn rows
      tn  = tanh(psum_L)      (ACT)
      tns = tn * scale -> f32r (DVE)  -> DMA to attn[j, i]
      PE-transpose tns 128x128 blocks -> Tb (attn^T blocks, AV lhsT)
      psum_av_j += Tb_s.T @ xn_s         # accumulate over all m
    out_j = psum_av_j + xres_j -> DMA
"""

import math

import numpy as np

import concourse.bacc as bacc
import concourse.bass as bass
import concourse.mybir as mybir
import concourse.tile as tile
from concourse import masks
from concourse.bass_utils import run_bass_kernel_spmd

F32 = mybir.dt.float32
F32R = mybir.dt.float32r

B, N, DX, DE = 4, 4096, 256, 128
D = DX + DE            # 384
NL = N // 2            # 2048 local rows per core
SCALE = D ** -0.5
N_CORES = 8


def round_f32r(a: np.ndarray) -> np.ndarray:
    """Round-to-nearest-even fp32 -> fp32r (11-bit mantissa, low 12 bits 0)."""
    u = np.ascontiguousarray(a, dtype=np.float32).view(np.uint32)
    low = u & np.uint32(0xFFF)
    base = u & np.uint32(0xFFFFF000)
    lsb = (u >> np.uint32(12)) & np.uint32(1)
    roundup = (low > 0x800) | ((low == 0x800) & (lsb == 1))
    out = base + (roundup.astype(np.uint32) << np.uint32(12))
    return out.view(np.float32)


def build():
    nc = bacc.Bacc("TRN2", target_bir_lowering=False, debug=False,
                   num_devices=N_CORES)

    xeT_ext = nc.declare_dram_parameter("xeT", [D, N], F32R, isOutput=False)
    xeTl_ext = nc.declare_dram_parameter("xeTl", [D, NL], F32R, isOutput=False)
    xn_ext = nc.declare_dram_parameter("xn", [N, DX], F32R, isOutput=False)
    xres_ext = nc.declare_dram_parameter("xres", [NL, DX], F32R, isOutput=False)
    wqT_ext = nc.declare_dram_parameter("wqT", [D, D], F32R, isOutput=False)
    wkT_ext = nc.declare_dram_parameter("wkT", [D, D], F32R, isOutput=False)
    attn_ext = nc.declare_dram_parameter("attn", [NL, N], F32, isOutput=True)
    out_ext = nc.declare_dram_parameter("out", [NL, DX], F32, isOutput=True)

    NJ = NL // 128        # 16 row tiles
    NI = N // 1024        # 4 m blocks
    NC3 = D // 128        # 3 d chunks

    with tile.TileContext(nc) as tc:
        with (
            tc.tile_pool(name="const", bufs=1) as const_pool,
            tc.tile_pool(name="resident", bufs=1) as res_pool,
            tc.tile_pool(name="xe_stream", bufs=6) as xe_pool,
            tc.tile_pool(name="work", bufs=3) as work_pool,
            tc.tile_pool(name="psum_l", bufs=2, space="PSUM") as pool_l,
            tc.tile_pool(name="psum_t", bufs=2, space="PSUM") as pool_t,
            tc.tile_pool(name="psum_av", bufs=2, space="PSUM") as pool_av,
        ):
            ident_f32 = const_pool.tile([128, 128], F32, name="ident_f32")
            masks.make_identity(nc, ident_f32[:])
            ident = const_pool.tile([128, 128], F32R, name="ident")
            nc.vector.tensor_copy(ident[:], ident_f32[:])

            # ---- resident tensors -------------------------------------
            wqT_sb = res_pool.tile([128, NC3 * D], F32R, name="wqT_sb")
            wkT_sb = res_pool.tile([128, NC3 * D], F32R, name="wkT_sb")
            # chunk c (rows c*128 of the [384, ...] dram tensor) at cols c*W
            nc.sync.dma_start(
                wqT_sb[:].rearrange("p (c d) -> p c d", c=NC3),
                wqT_ext.rearrange("(c p) d -> p c d", p=128))
            nc.sync.dma_start(
                wkT_sb[:].rearrange("p (c d) -> p c d", c=NC3),
                wkT_ext.rearrange("(c p) d -> p c d", p=128))

            xn_sb = res_pool.tile([128, (N // 128) * DX], F32R, name="xn_sb")
            for g in range(4):
                nc.sync.dma_start(
                    xn_sb[:, g * 8 * DX:(g + 1) * 8 * DX]
                    .rearrange("p (c t) -> p c t", c=8),
                    xn_ext[g * 1024:(g + 1) * 1024, :]
                    .rearrange("(c p) t -> p c t", p=128))
            xres_sb = res_pool.tile([128, NJ * DX], F32R, name="xres_sb")
            for g in range(2):
                nc.sync.dma_start(
                    xres_sb[:, g * 8 * DX:(g + 1) * 8 * DX]
                    .rearrange("p (c t) -> p c t", c=8),
                    xres_ext[g * 1024:(g + 1) * 1024, :]
                    .rearrange("(c p) t -> p c t", p=128))

            kT_sb = res_pool.tile([128, NC3 * N], F32R, name="kT_sb")
            qT_sb = res_pool.tile([128, NC3 * NL], F32R, name="qT_sb")

            # ---- projections: kT = WkT.T @ xeT, qT = WqT.T @ xeTl -----
            def project(src_ext, src_n, wT_sb, dst_sb):
                nblk = src_n // 1024
                for blk in range(nblk):
                    xet = [xe_pool.tile([128, 1024], F32R, name="xet",
                                        tag="xet") for _ in range(NC3)]
                    for c_in in range(NC3):
                        nc.sync.dma_start(
                            xet[c_in][:],
                            src_ext[c_in * 128:(c_in + 1) * 128,
                                    blk * 1024:(blk + 1) * 1024])
                    for c_out in range(NC3):
                        ps = pool_l.tile([128, 1024], F32, name="ps_proj",
                                         tag="psl")
                        for c_in in range(NC3):
                            lhs = wT_sb[:, c_in * D + c_out * 128:
                                        c_in * D + (c_out + 1) * 128]
                            for hh in range(2):
                                nc.tensor.matmul(
                                    ps[:, hh * 512:(hh + 1) * 512],
                                    lhs,
                                    xet[c_in][:, hh * 512:(hh + 1) * 512],
                                    start=(c_in == 0), stop=(c_in == NC3 - 1))
                        nc.vector.tensor_copy(
                            dst_sb[:, c_out * src_n + blk * 1024:
                                   c_out * src_n + (blk + 1) * 1024],
                            ps[:].bitcast(F32R))

            project(xeT_ext, N, wkT_sb, kT_sb)
            project(xeTl_ext, NL, wqT_sb, qT_sb)

            # ---- main loop -------------------------------------------
            for j in range(NJ):
                psum_av = pool_av.tile([128, DX], F32, name="psum_av")
                for i in range(NI):
                    psl = pool_l.tile([128, 1024], F32, name="psl", tag="psl")
                    for hh in range(2):
                        for c in range(NC3):
                            nc.tensor.matmul(
                                psl[:, hh * 512:(hh + 1) * 512],
                                qT_sb[:, c * NL + j * 128:
                                      c * NL + (j + 1) * 128],
                                kT_sb[:, c * N + i * 1024 + hh * 512:
                                      c * N + i * 1024 + (hh + 1) * 512],
                                start=(c == 0), stop=(c == NC3 - 1))
                    tn = work_pool.tile([128, 1024], F32, name="tn", tag="tn")
                    nc.scalar.activation(tn[:], psl[:],
                                         mybir.ActivationFunctionType.Tanh)
                    tns = work_pool.tile([128, 1024], F32R, name="tns",
                                         tag="tns")
                    nc.vector.tensor_scalar_mul(tns[:], tn[:], SCALE)
                    nc.sync.dma_start(
                        attn_ext[j * 128:(j + 1) * 128,
                                 i * 1024:(i + 1) * 1024],
                        tns[:].bitcast(F32))
                    for half in range(2):
                        pst = pool_t.tile([128, 512], F32, name="pst")
                        for s in range(4):
                            nc.tensor.matmul(
                                pst[:, s * 128:(s + 1) * 128].bitcast(F32R),
                                tns[:, half * 512 + s * 128:
                                    half * 512 + (s + 1) * 128],
                                ident[:],
                                is_transpose=True,
                                start=(s == 0), stop=(s == 3))
                        tb = work_pool.tile([128, 512], F32R, name="tb",
                                            tag="tb")
                        nc.vector.tensor_copy(tb[:], pst[:].bitcast(F32R))
                        for s in range(4):
                            mchunk = i * 8 + half * 4 + s
                            nc.tensor.matmul(
                                psum_av[:],
                                tb[:, s * 128:(s + 1) * 128],
                                xn_sb[:, mchunk * DX:(mchunk + 1) * DX],
                                start=(i == 0 and half == 0 and s == 0),
                                stop=(i == NI - 1 and half == 1 and s == 3))
                outt = work_pool.tile([128, DX], F32, name="outt", tag="outt")
                nc.vector.tensor_add(
                    outt[:], psum_av[:],
                    xres_sb[:, j * DX:(j + 1) * DX].bitcast(F32))
                nc.sync.dma_start(out_ext[j * 128:(j + 1) * 128, :], outt[:])

    nc.compile()
    return nc


_NC_CACHE = None


def _get_nc():
    global _NC_CACHE
    if _NC_CACHE is None:
        _NC_CACHE = build()
    return _NC_CACHE


def make_in_maps(x, e, Wq, Wk):
    x = np.asarray(x, dtype=np.float32)
    e = np.asarray(e, dtype=np.float32)
    wqT = round_f32r(np.ascontiguousarray(np.asarray(Wq, np.float32).T))
    wkT = round_f32r(np.ascontiguousarray(np.asarray(Wk, np.float32).T))
    in_maps = []
    for c in range(N_CORES):
        b, h = c // 2, c % 2
        xeT = round_f32r(np.ascontiguousarray(
            np.concatenate([x[b], e[b]], axis=-1).T))
        xn = round_f32r(np.ascontiguousarray(x[b]))
        in_maps.append({
            "xeT": xeT,
            "xeTl": np.ascontiguousarray(xeT[:, h * NL:(h + 1) * NL]),
            "xn": xn,
            "xres": np.ascontiguousarray(xn[h * NL:(h + 1) * NL]),
            "wqT": wqT,
            "wkT": wkT,
        })
    return in_maps


def assemble(results, x, pred_step):
    x = np.asarray(x, dtype=np.float32)
    attn = np.empty((B, N, N), dtype=np.float32)
    out = np.empty((B, N, DX), dtype=np.float32)
    for c in range(N_CORES):
        b, h = c // 2, c % 2
        attn[b, h * NL:(h + 1) * NL] = results[c]["attn"]
        out[b, h * NL:(h + 1) * NL] = results[c]["out"]
    # autoregressive tail (0.4% of FLOPs): cur = attn @ cur + cur
    extras = []
    cur = out[:, :, -1:]
    for _ in range(int(pred_step) - 1):
        cur = np.matmul(attn, cur) + cur
        extras.append(cur)
    if extras:
        out = np.concatenate([out] + extras, axis=-1)
    return out, attn


def kernel(x, e, Wq, Wk, pred_step):
    nc = _get_nc()
    in_maps = make_in_maps(x, e, Wq, Wk)
    res = run_bass_kernel_spmd(nc, in_maps, core_ids=list(range(N_CORES)))
    return assemble(res.results, x, pred_step)


# revision 9
# speedup vs baseline: 30955.3212x; 30955.3212x over previous
"""Trainium2 Bass kernel for nn_Attention_18433999635238 (attnT-primary).

Per core (batch b, row-half h):
  kT = WkT.T @ xeT  [384, 4096]      qT = WqT.T @ xeTl  [384, 2048]
  per (n-block i of 1024 [2], m-tile j of 128 [32]):
    psum_L[m 128, n 1024] = sum_c kT[:,m_j].T @ qT[:,n_i]   (f32r, 6 mm N=512)
    tnT  = tanh(psum_L) -> bf16       (ACT, one op)
    tnsT = tnT * scale  -> bf16 (DVE) -> DMA attnT[j*128, i*1024]
    avT[tc] += xn_j[:,tc*128:+128].T @ tnT     (bf16, 2 mm N=1024)
  at j end: outT rows = avT * scale + xresT -> DMA outT  [256, 2048]
Host: attn = attnT.T, out256 = outT.T  (layout transforms at unshard).
"""

import numpy as np

import concourse.bacc as bacc
import concourse.bass as bass
import concourse.mybir as mybir
import concourse.tile as tile
from concourse import masks
from concourse.bass_utils import run_bass_kernel_spmd

F32 = mybir.dt.float32
F32R = mybir.dt.float32r
BF16 = mybir.dt.bfloat16

B, N, DX, DE = 4, 4096, 256, 128
D = DX + DE            # 384
NL = N // 2            # 2048 local rows per core
SCALE = D ** -0.5
N_CORES = 8

NBLK = 512             # n-block width
NI = NL // NBLK        # 2 n-blocks
NJ = N // 128          # 32 m tiles
NC3 = D // 128         # 3 d chunks
NT = DX // 128         # 2 t chunks


def round_f32r(a: np.ndarray) -> np.ndarray:
    u = np.ascontiguousarray(a, dtype=np.float32).view(np.uint32)
    low = u & np.uint32(0xFFF)
    base = u & np.uint32(0xFFFFF000)
    lsb = (u >> np.uint32(12)) & np.uint32(1)
    roundup = (low > 0x800) | ((low == 0x800) & (lsb == 1))
    out = base + (roundup.astype(np.uint32) << np.uint32(12))
    return out.view(np.float32)


def _body(nc, tc, ext):
    (xeT_ext, xeTl_ext, xn_ext, xresT_ext, wqT_ext, wkT_ext,
     attnT_ext, outT_ext) = ext
    with (
        tc.tile_pool(name="resident", bufs=1) as res_pool,
        tc.tile_pool(name="xe_stream", bufs=6) as xe_pool,
        tc.tile_pool(name="work", bufs=3) as work_pool,
        tc.tile_pool(name="psum_l", bufs=6, space="PSUM") as pool_l,
        tc.tile_pool(name="psum_av", bufs=2, space="PSUM") as pool_av,
    ):
        # ---- resident tensors -----------------------------------------
        wqT_sb = res_pool.tile([128, NC3 * D], F32R, name="wqT_sb")
        wkT_sb = res_pool.tile([128, NC3 * D], F32R, name="wkT_sb")
        nc.sync.dma_start(
            wqT_sb[:].rearrange("p (c d) -> p c d", c=NC3),
            wqT_ext.rearrange("(c p) d -> p c d", p=128))
        nc.sync.dma_start(
            wkT_sb[:].rearrange("p (c d) -> p c d", c=NC3),
            wkT_ext.rearrange("(c p) d -> p c d", p=128))

        xn_sb = res_pool.tile([128, (N // 128) * DX], BF16, name="xn_sb")
        for g in range(4):
            nc.sync.dma_start(
                xn_sb[:, g * 8 * DX:(g + 1) * 8 * DX]
                .rearrange("p (c t) -> p c t", c=8),
                xn_ext[g * 1024:(g + 1) * 1024, :]
                .rearrange("(c p) t -> p c t", p=128))
        xresT_sb = res_pool.tile([128, NT * NL], F32R, name="xresT_sb")
        nc.sync.dma_start(
            xresT_sb[:].rearrange("p (c t) -> p c t", c=NT),
            xresT_ext.rearrange("(c p) t -> p c t", p=128))

        kT_sb = res_pool.tile([128, NC3 * N], F32R, name="kT_sb")
        qT_sb = res_pool.tile([128, NC3 * NL], F32R, name="qT_sb")

        def project(src_ext, src_n, wT_sb, dst_sb):
            nblk = src_n // 1024
            for blk in range(nblk):
                xet = [xe_pool.tile([128, 1024], F32R, name="xet",
                                    tag="xet") for _ in range(NC3)]
                for c_in in range(NC3):
                    nc.sync.dma_start(
                        xet[c_in][:],
                        src_ext[c_in * 128:(c_in + 1) * 128,
                                blk * 1024:(blk + 1) * 1024])
                for c_out in range(NC3):
                    for hh in range(2):
                        ps = pool_l.tile([128, 512], F32, name="ps_proj",
                                         tag="psl")
                        for c_in in range(NC3):
                            lhs = wT_sb[:, c_in * D + c_out * 128:
                                        c_in * D + (c_out + 1) * 128]
                            nc.tensor.matmul(
                                ps[:],
                                lhs,
                                xet[c_in][:, hh * 512:(hh + 1) * 512],
                                start=(c_in == 0), stop=(c_in == NC3 - 1))
                        nc.scalar.copy(
                            dst_sb[:, c_out * src_n + blk * 1024 + hh * 512:
                                   c_out * src_n + blk * 1024 + (hh + 1) * 512],
                            ps[:].bitcast(F32R))

        project(xeT_ext, N, wkT_sb, kT_sb)
        project(xeTl_ext, NL, wqT_sb, qT_sb)

        # ---- main loop: software-pipelined emission -------------------
        av_psums = {}

        def emit_logits_pair(s1, s2):
            # interleave the c-chunk matmuls of two steps so consecutive
            # PE matmuls hit different PSUM banks
            psls = []
            for _ in range(len([s for s in (s1, s2) if s])):
                psls.append(pool_l.tile([128, NBLK], F32, name="psl",
                                        tag="psl"))
            for c in range(NC3):
                for idx, st in enumerate([s for s in (s1, s2) if s]):
                    i, j = st
                    nc.tensor.matmul(
                        psls[idx][:],
                        kT_sb[:, c * N + j * 128:c * N + (j + 1) * 128],
                        qT_sb[:, c * NL + i * NBLK:
                              c * NL + i * NBLK + NBLK],
                        start=(c == 0), stop=(c == NC3 - 1))
            return psls

        def emit_consumers(i, j, psl):
            if j == 0:
                av_psums[i] = [pool_av.tile([128, NBLK], F32, name="psum_av")
                               for _ in range(NT)]
            tnT = work_pool.tile([128, NBLK], BF16, name="tnT", tag="tnT")
            nc.scalar.activation(tnT[:], psl[:],
                                 mybir.ActivationFunctionType.Tanh)
            tnsT = work_pool.tile([128, NBLK], BF16, name="tnsT", tag="tnsT")
            nc.vector.tensor_scalar_mul(tnsT[:], tnT[:], SCALE)
            nc.sync.dma_start(
                attnT_ext[j * 128:(j + 1) * 128, i * NBLK:(i + 1) * NBLK],
                tnsT[:])
            for t in range(NT):
                for hh in range(NBLK // 512):
                    nc.tensor.matmul(
                        av_psums[i][t][:, hh * 512:(hh + 1) * 512],
                        xn_sb[:, j * DX + t * 128:j * DX + (t + 1) * 128],
                        tnT[:, hh * 512:(hh + 1) * 512],
                        start=(j == 0), stop=(j == NJ - 1))
            if j == NJ - 1:
                for t in range(NT):
                    outtT = work_pool.tile([128, NBLK], F32, name="outtT",
                                           tag="outtT")
                    nc.vector.tensor_scalar_mul(
                        outtT[:], av_psums[i][t][:], SCALE)
                    nc.vector.tensor_add(
                        outtT[:], outtT[:],
                        xresT_sb[:, t * NL + i * NBLK:
                                 t * NL + (i + 1) * NBLK].bitcast(F32))
                    nc.sync.dma_start(
                        outT_ext[t * 128:(t + 1) * 128,
                                 i * NBLK:(i + 1) * NBLK],
                        outtT[:])
                del av_psums[i]

        import collections
        steps = [(i, j) for i in range(NI) for j in range(NJ)]
        pending = collections.deque()
        for t in range(0, len(steps), 2):
            s1 = steps[t]
            s2 = steps[t + 1] if t + 1 < len(steps) else None
            psls = emit_logits_pair(s1, s2)
            for st, psl in zip([s for s in (s1, s2) if s], psls):
                pending.append((*st, psl))
            while len(pending) > 2:
                emit_consumers(*pending.popleft())
        while pending:
            emit_consumers(*pending.popleft())


def build(reps=None):
    nc = bacc.Bacc("TRN2", target_bir_lowering=False, debug=False,
                   num_devices=N_CORES)
    ext = (
        nc.declare_dram_parameter("xeT", [D, N], F32R, isOutput=False),
        nc.declare_dram_parameter("xeTl", [D, NL], F32R, isOutput=False),
        nc.declare_dram_parameter("xn", [N, DX], BF16, isOutput=False),
        nc.declare_dram_parameter("xresT", [DX, NL], F32R, isOutput=False),
        nc.declare_dram_parameter("wqT", [D, D], F32R, isOutput=False),
        nc.declare_dram_parameter("wkT", [D, D], F32R, isOutput=False),
        nc.declare_dram_parameter("attnT", [N, NL], BF16, isOutput=True),
        nc.declare_dram_parameter("outT", [DX, NL], F32, isOutput=True),
    )
    with tile.TileContext(nc) as tc:
        if reps:
            with tc.For_i(0, reps, 1):
                _body(nc, tc, ext)
        else:
            _body(nc, tc, ext)
    nc.compile()
    return nc


_NC_CACHE = None


def _get_nc():
    global _NC_CACHE
    if _NC_CACHE is None:
        _NC_CACHE = build()
    return _NC_CACHE


def make_in_maps(x, e, Wq, Wk):
    import ml_dtypes
    x = np.asarray(x, dtype=np.float32)
    e = np.asarray(e, dtype=np.float32)
    wqT = round_f32r(np.ascontiguousarray(np.asarray(Wq, np.float32).T))
    wkT = round_f32r(np.ascontiguousarray(np.asarray(Wk, np.float32).T))
    in_maps = []
    for c in range(N_CORES):
        b, h = c // 2, c % 2
        xeT = round_f32r(np.ascontiguousarray(
            np.concatenate([x[b], e[b]], axis=-1).T))
        xn_bf16 = np.ascontiguousarray(x[b]).astype(ml_dtypes.bfloat16)
        xresT = round_f32r(np.ascontiguousarray(x[b][h * NL:(h + 1) * NL].T))
        in_maps.append({
            "xeT": xeT,
            "xeTl": np.ascontiguousarray(xeT[:, h * NL:(h + 1) * NL]),
            "xn": xn_bf16,
            "xresT": xresT,
            "wqT": wqT,
            "wkT": wkT,
        })
    return in_maps


def assemble(results, x, pred_step):
    attn = np.empty((B, N, N), dtype=np.float32)
    out = np.empty((B, N, DX), dtype=np.float32)
    for c in range(N_CORES):
        b, h = c // 2, c % 2
        aT = np.asarray(results[c]["attnT"]).astype(np.float32)   # [N, NL]
        attn[b, h * NL:(h + 1) * NL, :] = aT.T
        out[b, h * NL:(h + 1) * NL] = np.asarray(results[c]["outT"]).T
    extras = []
    cur = out[:, :, -1:]
    for _ in range(int(pred_step) - 1):
        cur = np.matmul(attn, cur) + cur
        extras.append(cur)
    if extras:
        out = np.concatenate([out] + extras, axis=-1)
    return out, attn


def kernel(x, e, Wq, Wk, pred_step):
    nc = _get_nc()
    in_maps = make_in_maps(x, e, Wq, Wk)
    res = run_bass_kernel_spmd(nc, in_maps, core_ids=list(range(N_CORES)))
    return assemble(res.results, x, pred_step)


# revision 11
# speedup vs baseline: 31687.5835x; 1.0237x over previous
"""Trainium2 Bass kernel for nn_Attention_18433999635238 (attnT-primary).

Per core (batch b, row-half h):
  kT = WkT.T @ xeT  [384, 4096]      qT = WqT.T @ xeTl  [384, 2048]
  per (n-block i of 512 [4], m-tile j of 128 [32]):
    psum_L[m 128, n 512] = sum_c kT[:,m_j].T @ qT[:,n_i]    (f32r, 3 mm N=512)
    tnT  = tanh(psum_L) -> bf16       (ACT, one op)
    tnsT = tnT * scale  -> bf16 (DVE) -> DMA attnT[j*128, i*512]
    avT[tc] += xn_j[:,tc*128:+128].T @ tnT     (bf16 stationary-xn, N=512)
  at j end: outT rows = avT * scale + xresT -> DMA outT  [256, 2048]
Host: attn = attnT.T, out256 = outT.T  (layout transforms at unshard).
"""

import numpy as np

import concourse.bacc as bacc
import concourse.mybir as mybir
import concourse.tile as tile
from concourse.bass_utils import run_bass_kernel_spmd

F32 = mybir.dt.float32
F32R = mybir.dt.float32r
BF16 = mybir.dt.bfloat16

B, N, DX, DE = 4, 4096, 256, 128
D = DX + DE            # 384
NL = N // 2            # 2048 local rows per core
SCALE = D ** -0.5
N_CORES = 8

NBLK = 512             # n-block width
NI = NL // NBLK        # 2 n-blocks
NJ = N // 128          # 32 m tiles
NC3 = D // 128         # 3 d chunks
NT = DX // 128         # 2 t chunks


def round_f32r(a: np.ndarray) -> np.ndarray:
    u = np.ascontiguousarray(a, dtype=np.float32).view(np.uint32)
    low = u & np.uint32(0xFFF)
    base = u & np.uint32(0xFFFFF000)
    lsb = (u >> np.uint32(12)) & np.uint32(1)
    roundup = (low > 0x800) | ((low == 0x800) & (lsb == 1))
    out = base + (roundup.astype(np.uint32) << np.uint32(12))
    return out.view(np.float32)


def _body(nc, tc, ext):
    (xeT_ext, xeTl_ext, xn_ext, xresT_ext, wqT_ext, wkT_ext,
     attnT_ext, outT_ext) = ext
    with (
        tc.tile_pool(name="resident", bufs=1) as res_pool,
        tc.tile_pool(name="xe_stream", bufs=6) as xe_pool,
        tc.tile_pool(name="work", bufs=4) as work_pool,
        tc.tile_pool(name="psum_l", bufs=6, space="PSUM") as pool_l,
        tc.tile_pool(name="psum_av", bufs=2, space="PSUM") as pool_av,
    ):
        # ---- resident tensors -----------------------------------------
        wqT_sb = res_pool.tile([128, NC3 * D], F32R, name="wqT_sb")
        wkT_sb = res_pool.tile([128, NC3 * D], F32R, name="wkT_sb")
        nc.sync.dma_start(
            wqT_sb[:].rearrange("p (c d) -> p c d", c=NC3),
            wqT_ext.rearrange("(c p) d -> p c d", p=128))
        nc.sync.dma_start(
            wkT_sb[:].rearrange("p (c d) -> p c d", c=NC3),
            wkT_ext.rearrange("(c p) d -> p c d", p=128))

        xn_sb = res_pool.tile([128, (N // 128) * DX], BF16, name="xn_sb")
        for g in range(4):
            nc.sync.dma_start(
                xn_sb[:, g * 8 * DX:(g + 1) * 8 * DX]
                .rearrange("p (c t) -> p c t", c=8),
                xn_ext[g * 1024:(g + 1) * 1024, :]
                .rearrange("(c p) t -> p c t", p=128))
        xresT_sb = res_pool.tile([128, NT * NL], F32R, name="xresT_sb")
        nc.sync.dma_start(
            xresT_sb[:].rearrange("p (c t) -> p c t", c=NT),
            xresT_ext.rearrange("(c p) t -> p c t", p=128))

        kT_sb = res_pool.tile([128, NC3 * N], F32R, name="kT_sb")
        qT_sb = res_pool.tile([128, NC3 * NL], F32R, name="qT_sb")

        def project(src_ext, src_n, wT_sb, dst_sb):
            nblk = src_n // 1024
            for blk in range(nblk):
                xet = [xe_pool.tile([128, 1024], F32R, name="xet",
                                    tag="xet") for _ in range(NC3)]
                for c_in in range(NC3):
                    nc.sync.dma_start(
                        xet[c_in][:],
                        src_ext[c_in * 128:(c_in + 1) * 128,
                                blk * 1024:(blk + 1) * 1024])
                for c_out in range(NC3):
                    for hh in range(2):
                        ps = pool_l.tile([128, 512], F32, name="ps_proj",
                                         tag="psl")
                        for c_in in range(NC3):
                            lhs = wT_sb[:, c_in * D + c_out * 128:
                                        c_in * D + (c_out + 1) * 128]
                            nc.tensor.matmul(
                                ps[:],
                                lhs,
                                xet[c_in][:, hh * 512:(hh + 1) * 512],
                                start=(c_in == 0), stop=(c_in == NC3 - 1))
                        nc.scalar.copy(
                            dst_sb[:, c_out * src_n + blk * 1024 + hh * 512:
                                   c_out * src_n + blk * 1024 + (hh + 1) * 512],
                            ps[:].bitcast(F32R))

        project(xeT_ext, N, wkT_sb, kT_sb)
        project(xeTl_ext, NL, wqT_sb, qT_sb)

        # ---- main loop: deep-interleaved emission ---------------------
        # PE stream per pair: Lc0(t) Lc0(t+1) AV0(t-2) Lc1(t) Lc1(t+1)
        # AV1(t-2) Lc2(t) Lc2(t+1) -- consecutive matmuls always hit
        # different PSUM banks and stationary reloads pipeline.
        av_psums = {}

        def alloc_psls(nsteps):
            return [pool_l.tile([128, NBLK], F32, name="psl", tag="psl")
                    for _ in range(nsteps)]

        def emit_logits_c(st, psl, c):
            i, j = st
            nc.tensor.matmul(
                psl[:],
                kT_sb[:, c * N + j * 128:c * N + (j + 1) * 128],
                qT_sb[:, c * NL + i * NBLK:c * NL + i * NBLK + NBLK],
                start=(c == 0), stop=(c == NC3 - 1))

        def emit_tanh(i, j, psl):
            if j == 0:
                av_psums[i] = [pool_av.tile([128, NBLK], F32, name="psum_av")
                               for _ in range(NT)]
            tnT = work_pool.tile([128, NBLK], BF16, name="tnT", tag="tnT")
            nc.scalar.activation(tnT[:], psl[:],
                                 mybir.ActivationFunctionType.Tanh)
            return tnT

        def emit_av(i, j, tnT, t):
            nc.tensor.matmul(
                av_psums[i][t][:],
                xn_sb[:, j * DX + t * 128:j * DX + (t + 1) * 128],
                tnT[:],
                start=(j == 0), stop=(j == NJ - 1))

        def emit_scale_dma(i, j, tnT):
            tnsT = work_pool.tile([128, NBLK], BF16, name="tnsT", tag="tnsT")
            nc.vector.tensor_scalar_mul(tnsT[:], tnT[:], SCALE)
            nc.sync.dma_start(
                attnT_ext[j * 128:(j + 1) * 128, i * NBLK:(i + 1) * NBLK],
                tnsT[:])

        def emit_out(i):
            for t in range(NT):
                outtT = work_pool.tile([128, NBLK], F32, name="outtT",
                                       tag="outtT")
                nc.vector.tensor_scalar_mul(
                    outtT[:], av_psums[i][t][:], SCALE)
                nc.vector.tensor_add(
                    outtT[:], outtT[:],
                    xresT_sb[:, t * NL + i * NBLK:
                             t * NL + (i + 1) * NBLK].bitcast(F32))
                nc.sync.dma_start(
                    outT_ext[t * 128:(t + 1) * 128,
                             i * NBLK:(i + 1) * NBLK],
                    outtT[:])
            del av_psums[i]

        import collections
        steps = [(i, j) for i in range(NI) for j in range(NJ)]
        ready = collections.deque()   # (i, j, tnT) with tanh emitted
        for t0 in range(0, len(steps), 2):
            pair = steps[t0:t0 + 2]
            psls = alloc_psls(len(pair))
            cons = []
            while ready and len(cons) < 2:
                cons.append(ready.popleft())
            for c in range(NC3):
                for st, psl in zip(pair, psls):
                    emit_logits_c(st, psl, c)
                if c < NT:
                    for cn in cons:
                        emit_av(cn[0], cn[1], cn[2], c)
            for cn in cons:
                emit_scale_dma(*cn)
                if cn[1] == NJ - 1:
                    emit_out(cn[0])
            for (st, psl) in zip(pair, psls):
                tnT = emit_tanh(st[0], st[1], psl)
                ready.append((st[0], st[1], tnT))
        while ready:
            cn = ready.popleft()
            for c in range(NT):
                emit_av(cn[0], cn[1], cn[2], c)
            emit_scale_dma(*cn)
            if cn[1] == NJ - 1:
                emit_out(cn[0])

def build(reps=None):
    nc = bacc.Bacc("TRN2", target_bir_lowering=False, debug=False,
                   num_devices=N_CORES)
    ext = (
        nc.declare_dram_parameter("xeT", [D, N], F32R, isOutput=False),
        nc.declare_dram_parameter("xeTl", [D, NL], F32R, isOutput=False),
        nc.declare_dram_parameter("xn", [N, DX], BF16, isOutput=False),
        nc.declare_dram_parameter("xresT", [DX, NL], F32R, isOutput=False),
        nc.declare_dram_parameter("wqT", [D, D], F32R, isOutput=False),
        nc.declare_dram_parameter("wkT", [D, D], F32R, isOutput=False),
        nc.declare_dram_parameter("attnT", [N, NL], BF16, isOutput=True),
        nc.declare_dram_parameter("outT", [DX, NL], F32, isOutput=True),
    )
    with tile.TileContext(nc) as tc:
        if reps:
            with tc.For_i(0, reps, 1):
                _body(nc, tc, ext)
        else:
            _body(nc, tc, ext)
    nc.compile()
    return nc


_NC_CACHE = None


def _get_nc():
    global _NC_CACHE
    if _NC_CACHE is None:
        _NC_CACHE = build()
    return _NC_CACHE


def make_in_maps(x, e, Wq, Wk):
    import ml_dtypes
    x = np.asarray(x, dtype=np.float32)
    e = np.asarray(e, dtype=np.float32)
    wqT = round_f32r(np.ascontiguousarray(np.asarray(Wq, np.float32).T))
    wkT = round_f32r(np.ascontiguousarray(np.asarray(Wk, np.float32).T))
    in_maps = []
    for c in range(N_CORES):
        b, h = c // 2, c % 2
        xeT = round_f32r(np.ascontiguousarray(
            np.concatenate([x[b], e[b]], axis=-1).T))
        xn_bf16 = np.ascontiguousarray(x[b]).astype(ml_dtypes.bfloat16)
        xresT = round_f32r(np.ascontiguousarray(x[b][h * NL:(h + 1) * NL].T))
        in_maps.append({
            "xeT": xeT,
            "xeTl": np.ascontiguousarray(xeT[:, h * NL:(h + 1) * NL]),
            "xn": xn_bf16,
            "xresT": xresT,
            "wqT": wqT,
            "wkT": wkT,
        })
    return in_maps


def assemble(results, x, pred_step):
    attn = np.empty((B, N, N), dtype=np.float32)
    out = np.empty((B, N, DX), dtype=np.float32)
    for c in range(N_CORES):
        b, h = c // 2, c % 2
        aT = np.asarray(results[c]["attnT"]).astype(np.float32)   # [N, NL]
        attn[b, h * NL:(h + 1) * NL, :] = aT.T
        out[b, h * NL:(h + 1) * NL] = np.asarray(results[c]["outT"]).T
    extras = []
    cur = out[:, :, -1:]
    for _ in range(int(pred_step) - 1):
        cur = np.matmul(attn, cur) + cur
        extras.append(cur)
    if extras:
        out = np.concatenate([out] + extras, axis=-1)
    return out, attn


def kernel(x, e, Wq, Wk, pred_step):
    nc = _get_nc()
    in_maps = make_in_maps(x, e, Wq, Wk)
    res = run_bass_kernel_spmd(nc, in_maps, core_ids=list(range(N_CORES)))
    return assemble(res.results, x, pred_step)


# revision 16
# speedup vs baseline: 51202.5154x; 1.6159x over previous
"""Trainium2 Bass kernel for nn_Attention_18433999635238 (attnT-primary).

Per core (batch b, row-half h):
  kT = WkT.T @ xeT  [384, 4096]      qT = WqT.T @ xeTl  [384, 2048]
  per (n-block i of 512 [4], m-tile j of 128 [32]):
    psum_L[m 128, n 512] = sum_c kT[:,m_j].T @ qT[:,n_i]    (f32r, 3 mm N=512)
    tnT  = tanh(psum_L) -> bf16       (ACT, one op)
    tnsT = tnT * scale  -> bf16 (DVE) -> DMA attnT[j*128, i*512]
    avT[tc] += xn_j[:,tc*128:+128].T @ tnT     (bf16 stationary-xn, N=512)
  at j end: outT rows = avT * scale + xresT -> DMA outT  [256, 2048]
Host: attn = attnT.T, out256 = outT.T  (layout transforms at unshard).
"""

import numpy as np

import concourse.bacc as bacc
import concourse.mybir as mybir
import concourse.tile as tile
from concourse.bass_utils import run_bass_kernel_spmd

F32 = mybir.dt.float32
F32R = mybir.dt.float32r
BF16 = mybir.dt.bfloat16

B, N, DX, DE = 4, 4096, 256, 128
D = DX + DE            # 384
NL = N // 2            # 2048 local rows per core
SCALE = D ** -0.5
N_CORES = 8

NBLK = 512             # n-block width
NI = NL // NBLK        # 2 n-blocks
NJ = N // 128          # 32 m tiles
NC3 = D // 128         # 3 d chunks
NT = DX // 128         # 2 t chunks


def round_f32r(a: np.ndarray) -> np.ndarray:
    u = np.ascontiguousarray(a, dtype=np.float32).view(np.uint32)
    low = u & np.uint32(0xFFF)
    base = u & np.uint32(0xFFFFF000)
    lsb = (u >> np.uint32(12)) & np.uint32(1)
    roundup = (low > 0x800) | ((low == 0x800) & (lsb == 1))
    out = base + (roundup.astype(np.uint32) << np.uint32(12))
    return out.view(np.float32)


def _body(nc, tc, ext):
    (xeT_ext, xeTl_ext, xn_ext, xresT_ext, wqT_ext, wkT_ext,
     attnT_ext, outT_ext) = ext
    with (
        tc.tile_pool(name="resident", bufs=1) as res_pool,
        tc.tile_pool(name="xe_stream", bufs=6) as xe_pool,
        tc.tile_pool(name="work", bufs=4) as work_pool,
        tc.tile_pool(name="psum_l", bufs=6, space="PSUM") as pool_l,
        tc.tile_pool(name="psum_av", bufs=2, space="PSUM") as pool_av,
    ):
        # ---- resident tensors -----------------------------------------
        wqT_sb = res_pool.tile([128, NC3 * D], F32R, name="wqT_sb")
        wkT_sb = res_pool.tile([128, NC3 * D], F32R, name="wkT_sb")
        nc.sync.dma_start(
            wqT_sb[:].rearrange("p (c d) -> p c d", c=NC3),
            wqT_ext.rearrange("(c p) d -> p c d", p=128))
        nc.sync.dma_start(
            wkT_sb[:].rearrange("p (c d) -> p c d", c=NC3),
            wkT_ext.rearrange("(c p) d -> p c d", p=128))

        kT_sb = res_pool.tile([128, NC3 * N], F32R, name="kT_sb")
        qT_sb = res_pool.tile([128, NC3 * NL], F32R, name="qT_sb")

        def project(src_ext, src_n, wT_sb, dst_sb):
            nblk = src_n // 1024
            for blk in range(nblk):
                xet = [xe_pool.tile([128, 1024], F32R, name="xet",
                                    tag="xet") for _ in range(NC3)]
                for c_in in range(NC3):
                    nc.sync.dma_start(
                        xet[c_in][:],
                        src_ext[c_in * 128:(c_in + 1) * 128,
                                blk * 1024:(blk + 1) * 1024])
                groups = [(c_out, hh) for c_out in range(NC3)
                          for hh in range(2)]
                for g0 in range(0, len(groups), 2):
                    pair = groups[g0:g0 + 2]
                    pss = [pool_l.tile([128, 512], F32, name="ps_proj",
                                       tag="psl") for _ in pair]
                    for c_in in range(NC3):
                        for (c_out, hh), ps in zip(pair, pss):
                            lhs = wT_sb[:, c_in * D + c_out * 128:
                                        c_in * D + (c_out + 1) * 128]
                            nc.tensor.matmul(
                                ps[:],
                                lhs,
                                xet[c_in][:, hh * 512:(hh + 1) * 512],
                                start=(c_in == 0), stop=(c_in == NC3 - 1))
                    for (c_out, hh), ps in zip(pair, pss):
                        nc.scalar.copy(
                            dst_sb[:, c_out * src_n + blk * 1024 + hh * 512:
                                   c_out * src_n + blk * 1024 + (hh + 1) * 512],
                            ps[:].bitcast(F32R))

        project(xeT_ext, N, wkT_sb, kT_sb)
        project(xeTl_ext, NL, wqT_sb, qT_sb)

        # xn/xresT are only consumed by the AV/out stages -- load them
        # after the projection stream so the first matmuls aren't queued
        # behind 5MB of resident-tensor DMA traffic.
        xn_sb = res_pool.tile([128, (N // 128) * DX], BF16, name="xn_sb")
        for g in range(4):
            nc.sync.dma_start(
                xn_sb[:, g * 8 * DX:(g + 1) * 8 * DX]
                .rearrange("p (c t) -> p c t", c=8),
                xn_ext[g * 1024:(g + 1) * 1024, :]
                .rearrange("(c p) t -> p c t", p=128))
        xresT_sb = res_pool.tile([128, NT * NL], F32R, name="xresT_sb")
        nc.sync.dma_start(
            xresT_sb[:].rearrange("p (c t) -> p c t", c=NT),
            xresT_ext.rearrange("(c p) t -> p c t", p=128))



        # ---- main loop: deep-interleaved emission ---------------------
        # PE stream per pair: Lc0(t) Lc0(t+1) AV0(t-2) Lc1(t) Lc1(t+1)
        # AV1(t-2) Lc2(t) Lc2(t+1) -- consecutive matmuls always hit
        # different PSUM banks and stationary reloads pipeline.
        av_psums = {}

        def alloc_psls(nsteps):
            return [pool_l.tile([128, NBLK], F32, name="psl", tag="psl")
                    for _ in range(nsteps)]

        def emit_logits_c(st, psl, c):
            i, j = st
            nc.tensor.matmul(
                psl[:],
                kT_sb[:, c * N + j * 128:c * N + (j + 1) * 128],
                qT_sb[:, c * NL + i * NBLK:c * NL + i * NBLK + NBLK],
                start=(c == 0), stop=(c == NC3 - 1))

        def emit_tanh(i, j, psl):
            if j == 0:
                av_psums[i] = [pool_av.tile([128, NBLK], F32, name="psum_av")
                               for _ in range(NT)]
            tnT = work_pool.tile([128, NBLK], BF16, name="tnT", tag="tnT")
            nc.scalar.activation(tnT[:], psl[:],
                                 mybir.ActivationFunctionType.Tanh)
            return tnT

        def emit_av(i, j, tnT, t):
            nc.tensor.matmul(
                av_psums[i][t][:],
                xn_sb[:, j * DX + t * 128:j * DX + (t + 1) * 128],
                tnT[:],
                start=(j == 0), stop=(j == NJ - 1))

        def emit_scale_dma(i, j, tnT):
            tnsT = work_pool.tile([128, NBLK], BF16, name="tnsT", tag="tnsT")
            nc.vector.tensor_scalar_mul(tnsT[:], tnT[:], SCALE)
            nc.sync.dma_start(
                attnT_ext[j * 128:(j + 1) * 128, i * NBLK:(i + 1) * NBLK],
                tnsT[:])

        def emit_out(i):
            for t in range(NT):
                outtT = work_pool.tile([128, NBLK], F32, name="outtT",
                                       tag="outtT")
                nc.vector.tensor_scalar_mul(
                    outtT[:], av_psums[i][t][:], SCALE)
                nc.vector.tensor_add(
                    outtT[:], outtT[:],
                    xresT_sb[:, t * NL + i * NBLK:
                             t * NL + (i + 1) * NBLK].bitcast(F32))
                nc.sync.dma_start(
                    outT_ext[t * 128:(t + 1) * 128,
                             i * NBLK:(i + 1) * NBLK],
                    outtT[:])
            del av_psums[i]

        import collections
        steps = [(i, j) for i in range(NI) for j in range(NJ)]
        ready = collections.deque()   # (i, j, tnT) with tanh emitted
        for t0 in range(0, len(steps), 2):
            pair = steps[t0:t0 + 2]
            psls = alloc_psls(len(pair))
            cons = []
            while ready and len(cons) < 2:
                cons.append(ready.popleft())
            for c in range(NC3):
                for st, psl in zip(pair, psls):
                    emit_logits_c(st, psl, c)
                if c < NT:
                    for cn in cons:
                        emit_av(cn[0], cn[1], cn[2], c)
            for cn in cons:
                emit_scale_dma(*cn)
                if cn[1] == NJ - 1:
                    emit_out(cn[0])
            for (st, psl) in zip(pair, psls):
                tnT = emit_tanh(st[0], st[1], psl)
                ready.append((st[0], st[1], tnT))
        while ready:
            cn = ready.popleft()
            for c in range(NT):
                emit_av(cn[0], cn[1], cn[2], c)
            emit_scale_dma(*cn)
            if cn[1] == NJ - 1:
                emit_out(cn[0])

def build(reps=None):
    nc = bacc.Bacc("TRN2", target_bir_lowering=False, debug=False,
                   num_devices=N_CORES)
    ext = (
        nc.declare_dram_parameter("xeT", [D, N], F32R, isOutput=False),
        nc.declare_dram_parameter("xeTl", [D, NL], F32R, isOutput=False),
        nc.declare_dram_parameter("xn", [N, DX], BF16, isOutput=False),
        nc.declare_dram_parameter("xresT", [DX, NL], F32R, isOutput=False),
        nc.declare_dram_parameter("wqT", [D, D], F32R, isOutput=False),
        nc.declare_dram_parameter("wkT", [D, D], F32R, isOutput=False),
        nc.declare_dram_parameter("attnT", [N, NL], BF16, isOutput=True),
        nc.declare_dram_parameter("outT", [DX, NL], F32, isOutput=True),
    )
    with tile.TileContext(nc) as tc:
        if reps:
            import os as _os
            hints = ()
            if _os.environ.get("LOOP_HINTS"):
                hints = (mybir.EngineType.PE, mybir.EngineType.Activation,
                         mybir.EngineType.DVE, mybir.EngineType.SP,
                         mybir.EngineType.Pool)
            with tc.For_i(0, reps, 1, hint_engines=hints):
                _body(nc, tc, ext)
        else:
            _body(nc, tc, ext)
    nc.compile()
    return nc


_NC_CACHE = None


def _get_nc():
    global _NC_CACHE
    if _NC_CACHE is None:
        _NC_CACHE = build()
    return _NC_CACHE


def make_in_maps(x, e, Wq, Wk):
    import ml_dtypes
    x = np.asarray(x, dtype=np.float32)
    e = np.asarray(e, dtype=np.float32)
    wqT = round_f32r(np.ascontiguousarray(np.asarray(Wq, np.float32).T))
    wkT = round_f32r(np.ascontiguousarray(np.asarray(Wk, np.float32).T))
    in_maps = []
    for c in range(N_CORES):
        b, h = c // 2, c % 2
        xeT = round_f32r(np.ascontiguousarray(
            np.concatenate([x[b], e[b]], axis=-1).T))
        xn_bf16 = np.ascontiguousarray(x[b]).astype(ml_dtypes.bfloat16)
        xresT = round_f32r(np.ascontiguousarray(x[b][h * NL:(h + 1) * NL].T))
        in_maps.append({
            "xeT": xeT,
            "xeTl": np.ascontiguousarray(xeT[:, h * NL:(h + 1) * NL]),
            "xn": xn_bf16,
            "xresT": xresT,
            "wqT": wqT,
            "wkT": wkT,
        })
    return in_maps


def assemble(results, x, pred_step):
    attn = np.empty((B, N, N), dtype=np.float32)
    out = np.empty((B, N, DX), dtype=np.float32)
    for c in range(N_CORES):
        b, h = c // 2, c % 2
        aT = np.asarray(results[c]["attnT"]).astype(np.float32)   # [N, NL]
        attn[b, h * NL:(h + 1) * NL, :] = aT.T
        out[b, h * NL:(h + 1) * NL] = np.asarray(results[c]["outT"]).T
    extras = []
    cur = out[:, :, -1:]
    for _ in range(int(pred_step) - 1):
        cur = np.matmul(attn, cur) + cur
        extras.append(cur)
    if extras:
        out = np.concatenate([out] + extras, axis=-1)
    return out, attn


def kernel(x, e, Wq, Wk, pred_step):
    nc = _get_nc()
    in_maps = make_in_maps(x, e, Wq, Wk)
    res = run_bass_kernel_spmd(nc, in_maps, core_ids=list(range(N_CORES)))
    return assemble(res.results, x, pred_step)
